# revision 18
# baseline (speedup 1.0000x reference)
"""Trainium2 Bass kernel for dual-softmax cosine-similarity attention.

Per batch b (L=512, D=768):
    pn = p / ||p||,  qn = q / ||q||           (L2 over D)
    S  = pn @ qn^T                            [L, L]
    out_p = softmax(S, axis=1) @ q            [L, D]
    out_q = softmax(S, axis=0)^T-style: W[i,j] = exp(S[i,j])/colsum_j,
            out_q = W @ p                     [L, D]

B=64 fp32 in/out. Data-parallel over B across 8 cores (8 batches/core).

Device-side layout (everything in the S^T frame so no on-chip
transposes are needed):
    G^T[j,i] = sum_d qt[d,j] pt[d,i]   (PE; qt/pt are host-normalized,
               scaled by 16 and shipped fp8e4 -> DoubleRow matmuls)
    E^T      = exp(G^T / 256), colsum[j] = sum_i E^T   (ACT, fused accum)
    F        = E^T * (1/colsum[j])     (DVE per-partition scalar)
    out_p[i,:] = (E^T.T @ [q | 1]) * 1/rowsum[i]
               (ones-column appended to q on host gives rowsum[i] as
                column 768 of the same PSUM tile; scale applied at the
                PSUM->SBUF evacuation, DVE tensor_scalar)
    out_q[i,:] = F.T @ p               (ACT copy evacuation)
Softmax max-subtraction is skipped: S entries are cosines in [-1,1].
Outputs are shipped bf16 and upcast on host.
"""

import numpy as np
import ml_dtypes

B, L, D = 64, 512, 768
N_CORES = 8
BPC = B // N_CORES  # batches per core
LT = L // 128  # 4
DT = D // 128  # 6

S_DT = "float8e4"  # similarity-matmul operand dtype: "float8e4" | "bfloat16"
SCALE = 16.0  # host scales normalized rows by this; exp scale = 1/SCALE^2
EPS = 1e-8

_cache = {}


def _np_s_dt():
    return ml_dtypes.float8_e4m3 if S_DT == "float8e4" else ml_dtypes.bfloat16


def _build(s_dt_name, bpc=BPC):
    import concourse.tile as tile
    import concourse.mybir as mybir
    from concourse import bacc

    f32 = mybir.dt.float32
    bf16 = mybir.dt.bfloat16
    SD = getattr(mybir.dt, s_dt_name)
    AF = mybir.ActivationFunctionType
    DR = (
        mybir.MatmulPerfMode.DoubleRow if s_dt_name.startswith("float8") else None
    )

    nc = bacc.Bacc("TRN2", target_bir_lowering=False, debug=False)

    # Host-packed layouts: partition dim second so every partition's slice
    # is one long contiguous run (128 descriptors per DMA instead of 768).
    p_nat = nc.dram_tensor(
        "p_nat", [bpc, 128, LT, D], bf16, kind="ExternalInput"
    ).ap()
    q_nat = nc.dram_tensor(
        "q_nat", [bpc, 128, LT, D + 1], bf16, kind="ExternalInput"
    ).ap()
    p_t = nc.dram_tensor("p_t", [bpc, 128, DT, L], SD, kind="ExternalInput").ap()
    q_t = nc.dram_tensor("q_t", [bpc, 128, DT, L], SD, kind="ExternalInput").ap()
    out_p = nc.dram_tensor("out_p", [bpc, L, D], bf16, kind="ExternalOutput").ap()
    out_q = nc.dram_tensor("out_q", [bpc, L, D], bf16, kind="ExternalOutput").ap()

    with tile.TileContext(nc) as tc:
        with (
            tc.tile_pool(name="inp", bufs=5) as inp,
            tc.tile_pool(name="ew", bufs=2) as ew,
            tc.tile_pool(name="small", bufs=2) as small,
            tc.tile_pool(name="outs", bufs=12) as outs,
            tc.tile_pool(name="g_ps", bufs=2, space="PSUM") as g_ps,
            tc.tile_pool(name="o_ps", bufs=3, space="PSUM") as o_ps,
        ):
            state = {}

            def emit_load_t(b):
                pt = inp.tile([128, DT, L], SD, tag="pt", name=f"pt{b}")
                qt = inp.tile([128, DT, L], SD, tag="qt", name=f"qt{b}")
                nc.sync.dma_start(qt, q_t[b])
                # pt goes out on the ACT HWDGE ring so the two similarity
                # operands transfer in parallel
                nc.scalar.dma_start(pt, p_t[b])
                state.setdefault(b, {}).update(pt=pt, qt=qt)

            def emit_load_nat(b):
                pn = inp.tile([128, LT, D], bf16, tag="pn", name=f"pn{b}")
                qa = inp.tile([128, LT, D + 1], bf16, tag="qa", name=f"qa{b}")
                nc.sync.dma_start(qa, q_nat[b])
                nc.scalar.dma_start(pn, p_nat[b])
                state.setdefault(b, {}).update(pn=pn, qa=qa)

            def emit_gexp_jts(b, jts):
                st = state[b]
                pt, qt = st["pt"], st["qt"]
                if "et" not in st:
                    st["et"] = ew.tile([128, LT, L], bf16, tag="et", name=f"et{b}")
                    st["f"] = ew.tile([128, LT, L], bf16, tag="f", name=f"f{b}")
                    st["cs"] = small.tile([128, LT], f32, tag="cs", name=f"cs{b}")
                    st["rc"] = small.tile([128, LT], f32, tag="rc", name=f"rc{b}")
                et, f, colsum, rcol = st["et"], st["f"], st["cs"], st["rc"]
                for jt in jts:
                    gp = g_ps.tile([128, L], f32, tag="g", name=f"g{b}_{jt}")
                    jj = slice(jt * 128, (jt + 1) * 128)
                    if DR is not None:
                        for kk in range(DT // 2):
                            nc.tensor.matmul(
                                gp,
                                lhsT=qt[:, 2 * kk : 2 * kk + 2, jj],
                                rhs=pt[:, 2 * kk : 2 * kk + 2, :],
                                start=(kk == 0),
                                stop=(kk == DT // 2 - 1),
                                perf_mode=DR,
                            )
                    else:
                        for kt in range(DT):
                            nc.tensor.matmul(
                                gp,
                                lhsT=qt[:, kt, jj],
                                rhs=pt[:, kt, :],
                                start=(kt == 0),
                                stop=(kt == DT - 1),
                            )
                    nc.scalar.activation(
                        et[:, jt, :],
                        gp,
                        AF.Exp,
                        scale=1.0 / (SCALE * SCALE),
                        accum_out=colsum[:, jt : jt + 1],
                    )
                    nc.vector.reciprocal(rcol[:, jt : jt + 1], colsum[:, jt : jt + 1])
                    nc.vector.tensor_scalar_mul(
                        f[:, jt, :], et[:, jt, :], rcol[:, jt : jt + 1]
                    )

            def emit_out_m(b, m):
                st = state[b]
                pn, qa, et, f = st["pn"], st["qa"], st["et"], st["f"]
                if True:
                    mm = slice(m * 128, (m + 1) * 128)
                    ps = o_ps.tile([128, 1024], f32, tag="o", name=f"op{b}_{m}")
                    for jt in range(LT):
                        nc.tensor.matmul(
                            ps[:, 0:512],
                            lhsT=et[:, jt, mm],
                            rhs=qa[:, jt, 0:512],
                            start=(jt == 0),
                            stop=(jt == LT - 1),
                        )
                    for jt in range(LT):
                        nc.tensor.matmul(
                            ps[:, 512 : D + 1],
                            lhsT=et[:, jt, mm],
                            rhs=qa[:, jt, 512 : D + 1],
                            start=(jt == 0),
                            stop=(jt == LT - 1),
                        )
                    rrec = small.tile([128, 1], f32, tag="rr", name=f"rr{b}_{m}")
                    nc.vector.reciprocal(rrec, ps[:, D : D + 1])
                    sbp = outs.tile([128, D], bf16, tag="sb", name=f"sbp{b}_{m}")
                    nc.vector.tensor_scalar_mul(sbp, ps[:, 0:D], rrec)
                    nc.sync.dma_start(out_p[b, mm, :], sbp)

                    ps2 = o_ps.tile([128, 1024], f32, tag="o", name=f"oq{b}_{m}")
                    for jt in range(LT):
                        nc.tensor.matmul(
                            ps2[:, 0:512],
                            lhsT=f[:, jt, mm],
                            rhs=pn[:, jt, 0:512],
                            start=(jt == 0),
                            stop=(jt == LT - 1),
                        )
                    for jt in range(LT):
                        nc.tensor.matmul(
                            ps2[:, 512:D],
                            lhsT=f[:, jt, mm],
                            rhs=pn[:, jt, 512:D],
                            start=(jt == 0),
                            stop=(jt == LT - 1),
                        )
                    sbq = outs.tile([128, D], bf16, tag="sb", name=f"sbq{b}_{m}")
                    nc.scalar.copy(sbq, ps2[:, 0:D])
                    nc.sync.dma_start(out_q[b, mm, :], sbq)

            emit_load_t(0)
            emit_load_t(1)
            emit_load_nat(0)
            emit_load_nat(1)
            emit_load_t(2)
            emit_load_nat(2)
            emit_load_t(3)
            emit_load_nat(3)
            for b in range(bpc):
                emit_gexp_jts(b, [0, 1])
                if b > 0:
                    emit_out_m(b - 1, 0)
                emit_gexp_jts(b, [2, 3])
                if b > 0:
                    for m in range(1, LT):
                        emit_out_m(b - 1, m)
                if b + 4 < bpc:
                    emit_load_t(b + 4)
                    emit_load_nat(b + 4)
            for m in range(LT):
                emit_out_m(bpc - 1, m)

    nc.compile()
    return nc


def _get_nc():
    key = ("v2", S_DT)
    if key not in _cache:
        _cache[key] = _build(S_DT)
    return _cache[key]


def kernel(p, q):
    from concourse.bass_utils import run_bass_kernel_spmd

    nc = _get_nc()
    p = np.asarray(p, dtype=np.float32)
    q = np.asarray(q, dtype=np.float32)
    sd = _np_s_dt()
    bf = ml_dtypes.bfloat16

    pn = p / np.maximum(np.linalg.norm(p, axis=-1, keepdims=True), EPS)
    qn = q / np.maximum(np.linalg.norm(q, axis=-1, keepdims=True), EPS)

    def pack_t(x):
        # [B, L, D] normalized -> [B, 128, DT, L] where [b, pp, kt, n] =
        # x[b, n, kt*128+pp] (contiguous per-partition runs for DMA)
        xt = (x * SCALE).transpose(0, 2, 1)  # [B, D, L]
        return np.ascontiguousarray(
            xt.reshape(B, DT, 128, L).transpose(0, 2, 1, 3)
        ).astype(sd)

    def pack_nat(x):
        # [B, L, W] -> [B, 128, LT, W] where [b, pp, t, d] = x[b, t*128+pp, d]
        w = x.shape[-1]
        return np.ascontiguousarray(
            x.reshape(B, LT, 128, w).transpose(0, 2, 1, 3)
        ).astype(bf)

    p_t_all = pack_t(pn)
    q_t_all = pack_t(qn)
    p_nat_all = pack_nat(p)
    ones = np.ones((B, L, 1), np.float32)
    q_nat_all = pack_nat(np.concatenate([q, ones], axis=-1))

    in_maps = []
    for c in range(N_CORES):
        sl = slice(c * BPC, (c + 1) * BPC)
        in_maps.append(
            {
                "p_nat": np.ascontiguousarray(p_nat_all[sl]),
                "q_nat": np.ascontiguousarray(q_nat_all[sl]),
                "p_t": np.ascontiguousarray(p_t_all[sl]),
                "q_t": np.ascontiguousarray(q_t_all[sl]),
            }
        )

    res = run_bass_kernel_spmd(nc, in_maps, core_ids=list(range(N_CORES)))
    _cache["last_result"] = res
    vec_att_p = np.concatenate([r["out_p"] for r in res.results], axis=0).astype(
        np.float32
    )
    vec_att_q = np.concatenate([r["out_q"] for r in res.results], axis=0).astype(
        np.float32
    )
    return vec_att_p, vec_att_q


if __name__ == "__main__":
    rng = np.random.default_rng(0)
    p = rng.standard_normal((B, L, D)).astype(np.float32)
    q = rng.standard_normal((B, L, D)).astype(np.float32)
    op, oq = kernel(p, q)
    print("shapes:", op.shape, oq.shape, op.dtype, oq.dtype)


# revision 19
# speedup vs baseline: 1.0595x; 1.0595x over previous
"""Trainium2 Bass kernel for dual-softmax cosine-similarity attention.

Per batch b (L=512, D=768):
    pn = p / ||p||,  qn = q / ||q||           (L2 over D)
    S  = pn @ qn^T                            [L, L]
    out_p = softmax(S, axis=1) @ q            [L, D]
    out_q = softmax(S, axis=0)^T-style: W[i,j] = exp(S[i,j])/colsum_j,
            out_q = W @ p                     [L, D]

B=64 fp32 in/out. Data-parallel over B across 8 cores (8 batches/core).

Device-side layout (everything in the S^T frame so no on-chip
transposes are needed):
    G^T[j,i] = sum_d qt[d,j] pt[d,i]   (PE; qt/pt are host-normalized,
               scaled by 16 and shipped fp8e4 -> DoubleRow matmuls)
    E^T      = exp(G^T / 256), colsum[j] = sum_i E^T   (ACT, fused accum)
    F        = E^T * (1/colsum[j])     (DVE per-partition scalar)
    out_p[i,:] = (E^T.T @ [q | 1]) * 1/rowsum[i]
               (ones-column appended to q on host gives rowsum[i] as
                column 768 of the same PSUM tile; scale applied at the
                PSUM->SBUF evacuation, DVE tensor_scalar)
    out_q[i,:] = F.T @ p               (ACT copy evacuation)
Softmax max-subtraction is skipped: S entries are cosines in [-1,1].
Outputs are shipped bf16 and upcast on host.
"""

import numpy as np
import ml_dtypes

B, L, D = 64, 512, 768
N_CORES = 8
BPC = B // N_CORES  # batches per core
LT = L // 128  # 4
DT = D // 128  # 6

S_DT = "float8e4"  # similarity-matmul operand dtype: "float8e4" | "bfloat16"
SCALE = 16.0  # host scales normalized rows by this; exp scale = 1/SCALE^2
EPS = 1e-8

_cache = {}


def _np_s_dt():
    return ml_dtypes.float8_e4m3 if S_DT == "float8e4" else ml_dtypes.bfloat16


def _build(s_dt_name, bpc=BPC):
    import concourse.tile as tile
    import concourse.mybir as mybir
    from concourse import bacc

    f32 = mybir.dt.float32
    bf16 = mybir.dt.bfloat16
    SD = getattr(mybir.dt, s_dt_name)
    AF = mybir.ActivationFunctionType
    DR = (
        mybir.MatmulPerfMode.DoubleRow if s_dt_name.startswith("float8") else None
    )

    nc = bacc.Bacc("TRN2", target_bir_lowering=False, debug=False)

    # Host-packed layouts: partition dim second so every partition's slice
    # is one long contiguous run (128 descriptors per DMA instead of 768).
    p_nat = nc.dram_tensor(
        "p_nat", [bpc, 128, LT, D], bf16, kind="ExternalInput"
    ).ap()
    q_nat = nc.dram_tensor(
        "q_nat", [bpc, 128, LT, D + 1], bf16, kind="ExternalInput"
    ).ap()
    p_t = nc.dram_tensor("p_t", [bpc, 128, DT, L], SD, kind="ExternalInput").ap()
    q_t = nc.dram_tensor("q_t", [bpc, 128, DT, L], SD, kind="ExternalInput").ap()
    out_p = nc.dram_tensor("out_p", [bpc, L, D], bf16, kind="ExternalOutput").ap()
    out_q = nc.dram_tensor("out_q", [bpc, L, D], bf16, kind="ExternalOutput").ap()

    with tile.TileContext(nc) as tc:
        with (
            tc.tile_pool(name="inp", bufs=5) as inp,
            tc.tile_pool(name="ew", bufs=2) as ew,
            tc.tile_pool(name="small", bufs=2) as small,
            tc.tile_pool(name="outs", bufs=12) as outs,
            tc.tile_pool(name="g_ps", bufs=2, space="PSUM") as g_ps,
            tc.tile_pool(name="o_ps", bufs=3, space="PSUM") as o_ps,
        ):
            state = {}

            def emit_load_t(b):
                pt = inp.tile([128, DT, L], SD, tag="pt", name=f"pt{b}")
                qt = inp.tile([128, DT, L], SD, tag="qt", name=f"qt{b}")
                nc.sync.dma_start(qt, q_t[b])
                # pt goes out on the ACT HWDGE ring so the two similarity
                # operands transfer in parallel
                nc.scalar.dma_start(pt, p_t[b])
                state.setdefault(b, {}).update(pt=pt, qt=qt)

            def emit_load_nat(b):
                pn = inp.tile([128, LT, D], bf16, tag="pn", name=f"pn{b}")
                qa = inp.tile([128, LT, D + 1], bf16, tag="qa", name=f"qa{b}")
                nc.sync.dma_start(qa, q_nat[b])
                nc.sync.dma_start(pn, p_nat[b])
                state.setdefault(b, {}).update(pn=pn, qa=qa)

            def emit_gexp_jts(b, jts):
                st = state[b]
                pt, qt = st["pt"], st["qt"]
                if "et" not in st:
                    st["et"] = ew.tile([128, LT, L], bf16, tag="et", name=f"et{b}")
                    st["f"] = ew.tile([128, LT, L], bf16, tag="f", name=f"f{b}")
                    st["cs"] = small.tile([128, LT], f32, tag="cs", name=f"cs{b}")
                    st["rc"] = small.tile([128, LT], f32, tag="rc", name=f"rc{b}")
                et, f, colsum, rcol = st["et"], st["f"], st["cs"], st["rc"]
                for jt in jts:
                    gp = g_ps.tile([128, L], f32, tag="g", name=f"g{b}_{jt}")
                    jj = slice(jt * 128, (jt + 1) * 128)
                    if DR is not None:
                        for kk in range(DT // 2):
                            nc.tensor.matmul(
                                gp,
                                lhsT=qt[:, 2 * kk : 2 * kk + 2, jj],
                                rhs=pt[:, 2 * kk : 2 * kk + 2, :],
                                start=(kk == 0),
                                stop=(kk == DT // 2 - 1),
                                perf_mode=DR,
                            )
                    else:
                        for kt in range(DT):
                            nc.tensor.matmul(
                                gp,
                                lhsT=qt[:, kt, jj],
                                rhs=pt[:, kt, :],
                                start=(kt == 0),
                                stop=(kt == DT - 1),
                            )
                    nc.scalar.activation(
                        et[:, jt, :],
                        gp,
                        AF.Exp,
                        scale=1.0 / (SCALE * SCALE),
                        accum_out=colsum[:, jt : jt + 1],
                    )
                    nc.vector.reciprocal(rcol[:, jt : jt + 1], colsum[:, jt : jt + 1])
                    nc.vector.tensor_scalar_mul(
                        f[:, jt, :], et[:, jt, :], rcol[:, jt : jt + 1]
                    )

            def emit_out_m(b, m):
                st = state[b]
                pn, qa, et, f = st["pn"], st["qa"], st["et"], st["f"]
                if True:
                    mm = slice(m * 128, (m + 1) * 128)
                    ps = o_ps.tile([128, 1024], f32, tag="o", name=f"op{b}_{m}")
                    for jt in range(LT):
                        nc.tensor.matmul(
                            ps[:, 0:512],
                            lhsT=et[:, jt, mm],
                            rhs=qa[:, jt, 0:512],
                            start=(jt == 0),
                            stop=(jt == LT - 1),
                        )
                    for jt in range(LT):
                        nc.tensor.matmul(
                            ps[:, 512 : D + 1],
                            lhsT=et[:, jt, mm],
                            rhs=qa[:, jt, 512 : D + 1],
                            start=(jt == 0),
                            stop=(jt == LT - 1),
                        )
                    rrec = small.tile([128, 1], f32, tag="rr", name=f"rr{b}_{m}")
                    nc.vector.reciprocal(rrec, ps[:, D : D + 1])
                    sbp = outs.tile([128, D], bf16, tag="sb", name=f"sbp{b}_{m}")
                    nc.vector.tensor_scalar_mul(sbp, ps[:, 0:D], rrec)
                    nc.sync.dma_start(out_p[b, mm, :], sbp)

                    ps2 = o_ps.tile([128, 1024], f32, tag="o", name=f"oq{b}_{m}")
                    for jt in range(LT):
                        nc.tensor.matmul(
                            ps2[:, 0:512],
                            lhsT=f[:, jt, mm],
                            rhs=pn[:, jt, 0:512],
                            start=(jt == 0),
                            stop=(jt == LT - 1),
                        )
                    for jt in range(LT):
                        nc.tensor.matmul(
                            ps2[:, 512:D],
                            lhsT=f[:, jt, mm],
                            rhs=pn[:, jt, 512:D],
                            start=(jt == 0),
                            stop=(jt == LT - 1),
                        )
                    sbq = outs.tile([128, D], bf16, tag="sb", name=f"sbq{b}_{m}")
                    nc.scalar.copy(sbq, ps2[:, 0:D])
                    nc.sync.dma_start(out_q[b, mm, :], sbq)

            emit_load_t(0)
            emit_load_t(1)
            emit_load_nat(0)
            emit_load_nat(1)
            emit_load_t(2)
            emit_load_nat(2)
            emit_load_t(3)
            emit_load_nat(3)
            for b in range(bpc):
                emit_gexp_jts(b, [0, 1])
                if b > 0:
                    emit_out_m(b - 1, 0)
                emit_gexp_jts(b, [2, 3])
                if b > 0:
                    for m in range(1, LT):
                        emit_out_m(b - 1, m)
                if b + 4 < bpc:
                    emit_load_t(b + 4)
                    emit_load_nat(b + 4)
            for m in range(LT):
                emit_out_m(bpc - 1, m)

    nc.compile()
    return nc


def _get_nc():
    key = ("v2", S_DT)
    if key not in _cache:
        _cache[key] = _build(S_DT)
    return _cache[key]


def kernel(p, q):
    from concourse.bass_utils import run_bass_kernel_spmd

    nc = _get_nc()
    p = np.asarray(p, dtype=np.float32)
    q = np.asarray(q, dtype=np.float32)
    sd = _np_s_dt()
    bf = ml_dtypes.bfloat16

    pn = p / np.maximum(np.linalg.norm(p, axis=-1, keepdims=True), EPS)
    qn = q / np.maximum(np.linalg.norm(q, axis=-1, keepdims=True), EPS)

    def pack_t(x):
        # [B, L, D] normalized -> [B, 128, DT, L] where [b, pp, kt, n] =
        # x[b, n, kt*128+pp] (contiguous per-partition runs for DMA)
        xt = (x * SCALE).transpose(0, 2, 1)  # [B, D, L]
        return np.ascontiguousarray(
            xt.reshape(B, DT, 128, L).transpose(0, 2, 1, 3)
        ).astype(sd)

    def pack_nat(x):
        # [B, L, W] -> [B, 128, LT, W] where [b, pp, t, d] = x[b, t*128+pp, d]
        w = x.shape[-1]
        return np.ascontiguousarray(
            x.reshape(B, LT, 128, w).transpose(0, 2, 1, 3)
        ).astype(bf)

    p_t_all = pack_t(pn)
    q_t_all = pack_t(qn)
    p_nat_all = pack_nat(p)
    ones = np.ones((B, L, 1), np.float32)
    q_nat_all = pack_nat(np.concatenate([q, ones], axis=-1))

    in_maps = []
    for c in range(N_CORES):
        sl = slice(c * BPC, (c + 1) * BPC)
        in_maps.append(
            {
                "p_nat": np.ascontiguousarray(p_nat_all[sl]),
                "q_nat": np.ascontiguousarray(q_nat_all[sl]),
                "p_t": np.ascontiguousarray(p_t_all[sl]),
                "q_t": np.ascontiguousarray(q_t_all[sl]),
            }
        )

    res = run_bass_kernel_spmd(nc, in_maps, core_ids=list(range(N_CORES)))
    _cache["last_result"] = res
    vec_att_p = np.concatenate([r["out_p"] for r in res.results], axis=0).astype(
        np.float32
    )
    vec_att_q = np.concatenate([r["out_q"] for r in res.results], axis=0).astype(
        np.float32
    )
    return vec_att_p, vec_att_q


if __name__ == "__main__":
    rng = np.random.default_rng(0)
    p = rng.standard_normal((B, L, D)).astype(np.float32)
    q = rng.standard_normal((B, L, D)).astype(np.float32)
    op, oq = kernel(p, q)
    print("shapes:", op.shape, oq.shape, op.dtype, oq.dtype)


# revision 22
# speedup vs baseline: 1.2223x; 1.1537x over previous
"""Trainium2 Bass kernel for dual-softmax cosine-similarity attention.

Per batch b (L=512, D=768):
    pn = p / ||p||,  qn = q / ||q||           (L2 over D)
    S  = pn @ qn^T                            [L, L]
    out_p = softmax(S, axis=1) @ q            [L, D]
    out_q = softmax(S, axis=0)^T-style: W[i,j] = exp(S[i,j])/colsum_j,
            out_q = W @ p                     [L, D]

B=64 fp32 in/out. Data-parallel over B across 8 cores (8 batches/core).

Device-side layout (everything in the S^T frame so no on-chip
transposes are needed):
    G^T[j,i] = sum_d qt[d,j] pt[d,i]   (PE; qt/pt are host-normalized,
               scaled by 16 and shipped fp8e4 -> DoubleRow matmuls)
    E^T      = exp(G^T / 256), colsum[j] = sum_i E^T   (ACT, fused accum)
    F        = E^T * (1/colsum[j])     (DVE per-partition scalar)
    out_p[i,:] = (E^T.T @ [q | 1]) * 1/rowsum[i]
               (ones-column appended to q on host gives rowsum[i] as
                column 768 of the same PSUM tile; scale applied at the
                PSUM->SBUF evacuation, DVE tensor_scalar)
    out_q[i,:] = F.T @ p               (ACT copy evacuation)
Softmax max-subtraction is skipped: S entries are cosines in [-1,1].
Outputs are shipped bf16 and upcast on host.
"""

import numpy as np
import ml_dtypes

B, L, D = 64, 512, 768
N_CORES = 8
BPC = B // N_CORES  # batches per core
LT = L // 128  # 4
DT = D // 128  # 6

S_DT = "float8e4"  # similarity-matmul operand dtype: "float8e4" | "bfloat16"
SCALE = 16.0  # host scales normalized rows by this; exp scale = 1/SCALE^2
EPS = 1e-8

_cache = {}


def _np_s_dt():
    return ml_dtypes.float8_e4m3 if S_DT == "float8e4" else ml_dtypes.bfloat16


def _build(s_dt_name, bpc=BPC):
    import concourse.tile as tile
    import concourse.mybir as mybir
    from concourse import bacc

    f32 = mybir.dt.float32
    bf16 = mybir.dt.bfloat16
    SD = getattr(mybir.dt, s_dt_name)
    AF = mybir.ActivationFunctionType
    DR = (
        mybir.MatmulPerfMode.DoubleRow if s_dt_name.startswith("float8") else None
    )

    nc = bacc.Bacc("TRN2", target_bir_lowering=False, debug=False)

    # Host-packed layouts: partition dim second so every partition's slice
    # is one long contiguous run (128 descriptors per DMA instead of 768).
    p_nat = nc.dram_tensor(
        "p_nat", [bpc, 128, LT, D], bf16, kind="ExternalInput"
    ).ap()
    q_nat = nc.dram_tensor(
        "q_nat", [bpc, 128, LT, D + 1], bf16, kind="ExternalInput"
    ).ap()
    p_t = nc.dram_tensor("p_t", [bpc, 128, DT, L], SD, kind="ExternalInput").ap()
    q_t = nc.dram_tensor("q_t", [bpc, 128, DT, L], SD, kind="ExternalInput").ap()
    out_p = nc.dram_tensor("out_p", [bpc, L, D], bf16, kind="ExternalOutput").ap()
    out_q = nc.dram_tensor("out_q", [bpc, L, D], bf16, kind="ExternalOutput").ap()

    with tile.TileContext(nc) as tc:
        with (
            tc.tile_pool(name="inp", bufs=5) as inp,
            tc.tile_pool(name="ew", bufs=2) as ew,
            tc.tile_pool(name="small", bufs=2) as small,
            tc.tile_pool(name="outs", bufs=12) as outs,
            tc.tile_pool(name="g_ps", bufs=2, space="PSUM") as g_ps,
            tc.tile_pool(name="o_ps", bufs=3, space="PSUM") as o_ps,
        ):
            state = {}

            def emit_load_t(b):
                pt = inp.tile([128, DT, L], SD, tag="pt", name=f"pt{b}")
                qt = inp.tile([128, DT, L], SD, tag="qt", name=f"qt{b}")
                nc.sync.dma_start(qt, q_t[b])
                # pt goes out on the ACT HWDGE ring so the two similarity
                # operands transfer in parallel
                nc.scalar.dma_start(pt, p_t[b])
                state.setdefault(b, {}).update(pt=pt, qt=qt)

            def emit_load_nat(b):
                pn = inp.tile([128, LT, D], bf16, tag="pn", name=f"pn{b}")
                qa = inp.tile([128, LT, D + 1], bf16, tag="qa", name=f"qa{b}")
                nc.sync.dma_start(qa, q_nat[b])
                nc.sync.dma_start(pn, p_nat[b])
                state.setdefault(b, {}).update(pn=pn, qa=qa)

            def emit_gexp_jts(b, jts):
                st = state[b]
                pt, qt = st["pt"], st["qt"]
                if "et" not in st:
                    st["et"] = ew.tile([128, LT, L], bf16, tag="et", name=f"et{b}")
                    st["f"] = ew.tile([128, LT, L], bf16, tag="f", name=f"f{b}")
                    st["cs"] = small.tile([128, LT], f32, tag="cs", name=f"cs{b}")
                    st["rc"] = small.tile([128, LT], f32, tag="rc", name=f"rc{b}")
                et, f, colsum, rcol = st["et"], st["f"], st["cs"], st["rc"]
                for jt in jts:
                    gp = g_ps.tile([128, L], f32, tag="g", name=f"g{b}_{jt}")
                    jj = slice(jt * 128, (jt + 1) * 128)
                    if DR is not None:
                        for kk in range(DT // 2):
                            nc.tensor.matmul(
                                gp,
                                lhsT=qt[:, 2 * kk : 2 * kk + 2, jj],
                                rhs=pt[:, 2 * kk : 2 * kk + 2, :],
                                start=(kk == 0),
                                stop=(kk == DT // 2 - 1),
                                perf_mode=DR,
                            )
                    else:
                        for kt in range(DT):
                            nc.tensor.matmul(
                                gp,
                                lhsT=qt[:, kt, jj],
                                rhs=pt[:, kt, :],
                                start=(kt == 0),
                                stop=(kt == DT - 1),
                            )
                    nc.scalar.activation(
                        et[:, jt, :],
                        gp,
                        AF.Exp,
                        scale=1.0 / (SCALE * SCALE),
                        accum_out=colsum[:, jt : jt + 1],
                    )
                    nc.vector.reciprocal(rcol[:, jt : jt + 1], colsum[:, jt : jt + 1])
                    nc.vector.tensor_scalar_mul(
                        f[:, jt, :], et[:, jt, :], rcol[:, jt : jt + 1]
                    )

            def emit_out_m(b, m, last=False):
                st = state[b]
                pn, qa, et, f = st["pn"], st["qa"], st["et"], st["f"]
                if True:
                    mm = slice(m * 128, (m + 1) * 128)
                    ps = o_ps.tile([128, 1024], f32, tag="o", name=f"op{b}_{m}")
                    for jt in range(LT):
                        nc.tensor.matmul(
                            ps[:, 0:512],
                            lhsT=et[:, jt, mm],
                            rhs=qa[:, jt, 0:512],
                            start=(jt == 0),
                            stop=(jt == LT - 1),
                        )
                    for jt in range(LT):
                        nc.tensor.matmul(
                            ps[:, 512 : D + 1],
                            lhsT=et[:, jt, mm],
                            rhs=qa[:, jt, 512 : D + 1],
                            start=(jt == 0),
                            stop=(jt == LT - 1),
                        )
                    rrec = small.tile([128, 1], f32, tag="rr", name=f"rr{b}_{m}")
                    nc.vector.reciprocal(rrec, ps[:, D : D + 1])
                    sbp = outs.tile([128, D], bf16, tag="sb", name=f"sbp{b}_{m}")
                    nc.vector.tensor_scalar_mul(sbp, ps[:, 0:D], rrec)
                    nc.sync.dma_start(out_p[b, mm, :], sbp)

                    ps2 = o_ps.tile([128, 1024], f32, tag="o", name=f"oq{b}_{m}")
                    for jt in range(LT):
                        nc.tensor.matmul(
                            ps2[:, 0:512],
                            lhsT=f[:, jt, mm],
                            rhs=pn[:, jt, 0:512],
                            start=(jt == 0),
                            stop=(jt == LT - 1),
                        )
                    for jt in range(LT):
                        nc.tensor.matmul(
                            ps2[:, 512:D],
                            lhsT=f[:, jt, mm],
                            rhs=pn[:, jt, 512:D],
                            start=(jt == 0),
                            stop=(jt == LT - 1),
                        )
                    sbq = outs.tile([128, D], bf16, tag="sb", name=f"sbq{b}_{m}")
                    if last:
                        # split the final evacuation so the last DMA is
                        # issued ~0.6us earlier (shortens the kernel tail)
                        h = D // 2
                        nc.scalar.copy(sbq[:, 0:h], ps2[:, 0:h])
                        nc.sync.dma_start(out_q[b, mm, 0:h], sbq[:, 0:h])
                        nc.scalar.copy(sbq[:, h:D], ps2[:, h:D])
                        nc.sync.dma_start(out_q[b, mm, h:D], sbq[:, h:D])
                    else:
                        nc.scalar.copy(sbq, ps2[:, 0:D])
                        nc.sync.dma_start(out_q[b, mm, :], sbq)

            emit_load_t(0)
            emit_load_t(1)
            emit_load_nat(0)
            emit_load_nat(1)
            emit_load_t(2)
            emit_load_nat(2)
            emit_load_t(3)
            emit_load_nat(3)
            for b in range(bpc):
                emit_gexp_jts(b, [0, 1])
                if b > 1:
                    emit_out_m(b - 1, 0)
                emit_gexp_jts(b, [2, 3])
                if b == 1:
                    # batch 0's nat inputs are still in flight when G(1)
                    # finishes; keep the PE on G work first
                    emit_out_m(0, 0)
                if b > 0:
                    for m in range(1, LT):
                        emit_out_m(b - 1, m)
                if b + 4 < bpc:
                    emit_load_t(b + 4)
                    emit_load_nat(b + 4)
            for m in range(LT):
                emit_out_m(bpc - 1, m, last=(m == LT - 1))

    nc.compile()
    return nc


def _get_nc():
    key = ("v2", S_DT)
    if key not in _cache:
        _cache[key] = _build(S_DT)
    return _cache[key]


def kernel(p, q):
    from concourse.bass_utils import run_bass_kernel_spmd

    nc = _get_nc()
    p = np.asarray(p, dtype=np.float32)
    q = np.asarray(q, dtype=np.float32)
    sd = _np_s_dt()
    bf = ml_dtypes.bfloat16

    pn = p / np.maximum(np.linalg.norm(p, axis=-1, keepdims=True), EPS)
    qn = q / np.maximum(np.linalg.norm(q, axis=-1, keepdims=True), EPS)

    def pack_t(x):
        # [B, L, D] normalized -> [B, 128, DT, L] where [b, pp, kt, n] =
        # x[b, n, kt*128+pp] (contiguous per-partition runs for DMA)
        xt = (x * SCALE).transpose(0, 2, 1)  # [B, D, L]
        return np.ascontiguousarray(
            xt.reshape(B, DT, 128, L).transpose(0, 2, 1, 3)
        ).astype(sd)

    def pack_nat(x):
        # [B, L, W] -> [B, 128, LT, W] where [b, pp, t, d] = x[b, t*128+pp, d]
        w = x.shape[-1]
        return np.ascontiguousarray(
            x.reshape(B, LT, 128, w).transpose(0, 2, 1, 3)
        ).astype(bf)

    p_t_all = pack_t(pn)
    q_t_all = pack_t(qn)
    p_nat_all = pack_nat(p)
    ones = np.ones((B, L, 1), np.float32)
    q_nat_all = pack_nat(np.concatenate([q, ones], axis=-1))

    in_maps = []
    for c in range(N_CORES):
        sl = slice(c * BPC, (c + 1) * BPC)
        in_maps.append(
            {
                "p_nat": np.ascontiguousarray(p_nat_all[sl]),
                "q_nat": np.ascontiguousarray(q_nat_all[sl]),
                "p_t": np.ascontiguousarray(p_t_all[sl]),
                "q_t": np.ascontiguousarray(q_t_all[sl]),
            }
        )

    res = run_bass_kernel_spmd(nc, in_maps, core_ids=list(range(N_CORES)))
    _cache["last_result"] = res
    vec_att_p = np.concatenate([r["out_p"] for r in res.results], axis=0).astype(
        np.float32
    )
    vec_att_q = np.concatenate([r["out_q"] for r in res.results], axis=0).astype(
        np.float32
    )
    return vec_att_p, vec_att_q


if __name__ == "__main__":
    rng = np.random.default_rng(0)
    p = rng.standard_normal((B, L, D)).astype(np.float32)
    q = rng.standard_normal((B, L, D)).astype(np.float32)
    op, oq = kernel(p, q)
    print("shapes:", op.shape, oq.shape, op.dtype, oq.dtype)
